# revision 38
# baseline (speedup 1.0000x reference)
"""Complex Gabor filter bank conv1d on 8 trn2 NeuronCores.

Problem: x [16, 1, 16000] f32 conv with 64 complex Gabor filters of length
402 -> out [16, 64, 15599] complex64.

Strategy:
- Data-parallel over batch: 2 rows per core, 8 cores, one shared NEFF (SPMD).
- Conv as matmul over the 402-tap contraction, split into THREE passes per
  512-wide output tile instead of four:
    * two fp16 matmuls for the center taps 65..320 (where the Gabor
      envelope carries nearly all its energy),
    * one fp8 e4m3 DoubleRow matmul for the outer taps 0..64 and 321..401.
  DoubleRow packs 2 fp8 weights per PE cell (contracts two 128-tap chunks
  in one pass, ~1.44x the bf16 rate at FD=512). The outer-tap envelope is
  tiny (|w| <= 3e-3), so e4m3 quantization there adds only ~3e-3 max rel
  err (validated vs float64 in numpy; gate is 2e-2). Weights are scaled by
  256 so outer weights land in e4m3's normal range; host divides it back.
- The fp16 "Hankel" buffer H16[p, i] = x16[row, p+i] is pre-built
  host-side and shipped as contiguous span tensors on the sync hardware
  DGE ring, both rows interleaved in conv need order (contiguous DMA fuses
  into large descriptors; strided rows pay a ~450ns/row descriptor cost).
  The fp8 Hankel H8 is derived ON-CHIP from H16 by per-span fp16->e4m3
  tensor copies (row 0 on vector, row 1 on scalar), emitted inside the
  conv loop so the in-order engines never block on a not-yet-landed span.
  This halves fill HBM traffic vs shipping H8 - the kernel is near the
  chip-level HBM roofline with all 8 cores running.
- All three matmul passes alias H16/H8 at different column offsets; the
  DoubleRow pair stride (320 taps apart, 16B-aligned as the ISA requires)
  is expressed in the 3D AP, so no special pair packing exists in memory.
- Conv groups alternate rows so neither row's fill ever idles the PE.
- Phase-separated DMA: the earliest store groups wait (explicit dep) for
  their row's fill to finish so the fill phase gets full DMA bandwidth;
  staging bufs=8 absorbs the deferral. Without this, fills+stores
  oversubscribe HBM mid-kernel and random cores stall 6-12us and get
  re-throttled by HAM.
- HAM warm-up: the PE clock-gate (K=4/8 -> half rate) releases only after
  ~3.4us of *sustained* activity and re-gates after idle gaps; a chain of
  dummy matmuls bridges boot->conv, and dep-free dummy matmuls after each
  early conv tile bridge short fill waits.
- PSUM->SBUF drain (fp32->fp16 cast) alternates vector/scalar engines;
  early store halves ride the scalar hw ring + gpsimd software ring, tail
  groups ride sync+scalar (hw rings flush fast at the end), as contiguous
  per-group blocks into a flat buffer the host reassembles to complex64.
"""

import sys

sys.path.insert(0, "/opt/trn_rl_repo")

import numpy as np
import ml_dtypes
import concourse.bass as bass
import concourse.bacc as bacc
import concourse.mybir as mybir
from concourse.tile import TileContext
from concourse.bass_utils import run_bass_kernel_spmd
from bass_rust import add_dep_helper as _add_dep

F32 = mybir.dt.float32
F16 = mybir.dt.float16
F8 = mybir.dt.float8e4
E4M3 = ml_dtypes.float8_e4m3
AF = mybir.ActivationFunctionType
DOUBLE_ROW = mybir.MatmulPerfMode.DoubleRow

N_CORES = 8
ROWS_PER_CORE = 2
T_IN = 16000
K_TAPS = 402
N_FILT = 64
T_OUT = T_IN - K_TAPS + 1  # 15599
TILE_N = 512
SCALE = 256.0         # weight scale so outer taps are e4m3-normal

TAP16_A = 65          # fp16 pass A covers taps 65..192
TAP16_B = 193         # fp16 pass B covers taps 193..320
TAP8_0 = 0            # fp8 chunk 0 covers taps 0..64 (rest zero-weighted)
TAP8_1 = 320          # fp8 chunk 1 covers taps 321..401 (tap 320 zeroed)

W8H = 15920           # H8 width:  last col = 15360 + 320 + 240 - 1 = 15919
W16H = W8H            # H16 matches so H8 is derived on-chip by fp16->fp8 cast
# fill chunks: conv group g's fp8 pass needs H8 cols <= 3072g+3392, which
# is covered by cast chunks 0..g+1; rows interleave so both rows stream
# concurrently on the sync hardware DGE ring
SPANS16 = [1536, 2560, 3072, 3072, 3072, W16H - 13312]
WU_MMS = 11           # dummy matmuls bridge the HAM clock-gate into conv

_CACHED_NC = None


def _tiles_of_row():
    tiles = []
    t0 = 0
    while t0 < T_OUT:
        tiles.append((t0, min(TILE_N, T_OUT - t0)))
        t0 += TILE_N
    return tiles


def _groups_of_row():
    """[(g0, width, [(t0, n), ...])] staging groups; short tail groups let
    the final drains+stores clear quickly after the last matmul."""
    tiles = _tiles_of_row()
    sizes = [6, 6, 6, 6, 4, 2, 1]
    chunks, i = [], 0
    for s in sizes:
        chunks.append(tiles[i : i + s])
        i += s
    assert i == len(tiles)
    groups = []
    for chunk in chunks:
        g0 = chunk[0][0]
        width = sum(n for _, n in chunk)
        groups.append((g0, width, chunk))
    return groups


def _build():
    nc = bacc.Bacc(target_bir_lowering=False)

    hk16 = [
        [
            nc.dram_tensor(f"hk16_{r}_{j}", [128, w], F16, kind="ExternalInput")
            for j, w in enumerate(SPANS16)
        ]
        for r in range(ROWS_PER_CORE)
    ]
    wt16 = nc.dram_tensor("wt16", [128, 256], F16, kind="ExternalInput")
    wt8 = nc.dram_tensor("wt8", [128, 256], F8, kind="ExternalInput")
    # conv output as contiguous per-group blocks: host reassembles
    o_fl = nc.dram_tensor(
        "o_fl", [ROWS_PER_CORE * 128 * T_OUT], F16, kind="ExternalOutput"
    )

    with TileContext(nc) as tc:
        with (
            tc.tile_pool(name="wp", bufs=1) as wp,       # weights + warmup srcs
            tc.tile_pool(name="hp", bufs=2) as hp,       # fp16 hankel buffers
            tc.tile_pool(name="h8p", bufs=2) as h8p,     # fp8 hankel buffers
            tc.tile_pool(name="sp", bufs=8) as sp,       # store staging
            tc.tile_pool(name="pp", bufs=7, space="PSUM") as pp,   # conv psum
            tc.tile_pool(name="wq", bufs=1, space="PSUM") as wq,   # warmup psum
        ):
            # warm-up sources (gpsimd is the first engine free after boot)
            wu_w = wp.tile([128, 128], F16, tag="wu_w")
            wu_r = wp.tile([128, TILE_N], F16, tag="wu_r")
            nc.gpsimd.memset(wu_w[:, :], 0.0)
            nc.gpsimd.memset(wu_r[:, :], 0.0)

            # weights first on the sync hw ring, then H16 chunks for both
            # rows interleaved in conv need order (engines serve jobs in
            # global trigger order, so trigger order equals need order)
            wt16_sb = wp.tile([128, 256], F16, tag="wt16_sb")
            wt8_sb = wp.tile([128, 256], F8, tag="wt8_sb")
            nc.sync.dma_start(wt16_sb[:, :], wt16.ap())
            nc.sync.dma_start(wt8_sb[:, :], wt8.ap())

            h16 = [
                hp.tile([128, W16H], F16, tag="H16", name=f"h16_{r}")
                for r in range(2)
            ]
            h8 = [
                h8p.tile([128, W8H], F8, tag="H8", name=f"h8_{r}")
                for r in range(2)
            ]

            # wake the gpsimd software DMA ring early (store halves ride it)
            gp_warm = wp.tile([128, 64], F16, tag="gp_warm")
            nc.gpsimd.dma_start(gp_warm[:, :], wt16.ap()[:, 0:64])

            # all fills on the sync hardware ring, interleaved in need order
            s16 = [0, 0]
            span_at = [[], []]  # per row: (start, width) of each span
            last_fill = [None, None]
            for j in range(len(SPANS16)):
                for r in range(2):
                    w = SPANS16[j]
                    a = s16[r]
                    dma = nc.sync.dma_start(
                        h16[r][:, a : a + w], hk16[r][j].ap()
                    )
                    last_fill[r] = dma
                    span_at[r].append((a, w))
                    s16[r] += w

            # HAM warm-up chain: no input deps, runs during the DMA fill and
            # bridges PE activity into the first conv matmul (the clock-gate
            # releases only after ~4us of *sustained* activity)
            wu_ps = wq.tile([128, TILE_N], F32, tag="wu_ps")
            for i in range(WU_MMS):
                nc.tensor.matmul(
                    wu_ps[:, :], wu_w[:, :], wu_r[:, :],
                    start=(i == 0), stop=(i == WU_MMS - 1),
                )

            # DoubleRow stationary AP: [128, 2, 128] over the wt8 tile
            w8ap = wt8_sb[:, :]
            w8_3d = bass.AP(
                w8ap.tensor, w8ap.offset, [list(w8ap.ap[0]), [128, 2], [1, 128]]
            )

            # fp16->fp8 casts derive H8 from H16 on-chip (saves 4MB/core of
            # fill DMA). They ride gpsimd - the only engine with slack mid-
            # conv (vector+scalar are saturated by PSUM drains; a cast
            # queued there causes PSUM backpressure that stalls the PE).
            # Chunks 0/1 are cast up front, chunk g+2 is emitted inside
            # conv group g so the in-order engine never blocks on a
            # not-yet-landed span.
            def cast_chunk(r, j):
                a, w = span_at[r][j]
                nc.gpsimd.tensor_copy(h8[r][:, a : a + w],
                                      h16[r][:, a : a + w])

            for r in range(2):
                cast_chunk(r, 0)
                cast_chunk(r, 1)

            # ---------------- conv ----------------
            # groups alternate rows so neither row's fill ever idles the PE
            groups = _groups_of_row()
            ti = 0
            for gi, (g0, gw, tiles) in enumerate(groups):
                for row in range(2):
                    h16r, h8r = h16[row], h8[row]
                    h8base = h8r[:, :]
                    stage = sp.tile([128, 3072], F16, tag="stage")
                    for tidx, (t0, n) in enumerate(tiles):
                        if tidx == 2 and gi + 2 < len(SPANS16):
                            cast_chunk(row, gi + 2)
                        ps = pp.tile([128, TILE_N], F32, tag="cv")
                        n_mm = n + (n & 1)  # keep moving-dim even
                        nc.tensor.matmul(
                            ps[:, :n_mm],
                            wt16_sb[:, 0:128],
                            h16r[:, t0 + TAP16_A : t0 + TAP16_A + n_mm],
                            start=True, stop=False,
                        )
                        nc.tensor.matmul(
                            ps[:, :n_mm],
                            wt16_sb[:, 128:256],
                            h16r[:, t0 + TAP16_B : t0 + TAP16_B + n_mm],
                            start=False, stop=False,
                        )
                        rhs8 = bass.AP(
                            h8base.tensor,
                            h8base.offset + t0,
                            [list(h8base.ap[0]), [TAP8_1, 2], [1, n_mm]],
                        )
                        nc.tensor.matmul(
                            ps[:, :n_mm], w8_3d, rhs8,
                            start=False, stop=True,
                            perf_mode=DOUBLE_ROW,
                        )
                        # early tiles: a dep-free dummy matmul keeps the PE
                        # active through short fill waits so the HAM window
                        # never resets (else the clock stays gated at half
                        # rate for the first ~10us on fill-jittered cores)
                        if gi < 2:
                            nc.tensor.matmul(
                                wu_ps[:, 0:128], wu_w[:, :], wu_r[:, 0:128],
                                start=True, stop=True,
                            )
                        off = t0 - g0
                        # drains 1:1 vector:scalar (vector also runs the
                        # fp16->fp8 casts; scalar also triggers stores)
                        if ti % 2 == 0:
                            nc.vector.tensor_copy(
                                stage[:, off : off + n], ps[:, :n]
                            )
                        else:
                            nc.scalar.activation(
                                stage[:, off : off + n], ps[:, :n], AF.Copy
                            )
                        ti += 1
                    # contiguous block store: [128, gw] at flat offset.
                    # Early groups split scalar hw ring / gpsimd software
                    # ring; tail groups ride the sync hw ring (idle by then,
                    # and the software ring is slow to flush at the end).
                    base = row * 128 * T_OUT + 128 * g0
                    lo = bass.AP(o_fl, base, [[gw, N_FILT], [1, gw]])
                    hi = bass.AP(
                        o_fl, base + N_FILT * gw, [[gw, N_FILT], [1, gw]]
                    )
                    if gi < 4:
                        st_a = nc.scalar.dma_start(lo, stage[0:N_FILT, :gw])
                        st_b = nc.gpsimd.dma_start(hi, stage[N_FILT:128, :gw])
                        if gi < 2:
                            # phase-separate DMA: the earliest stores wait
                            # for this row's fill to finish so fills get
                            # full bandwidth (staging bufs=8 covers it);
                            # later groups trigger after fills naturally
                            _add_dep(st_a.ins, last_fill[row].ins, sync=True,
                                     reason="stores behind fills")
                            _add_dep(st_b.ins, last_fill[row].ins, sync=True,
                                     reason="stores behind fills")
                    else:
                        nc.sync.dma_start(lo, stage[0:N_FILT, :gw])
                        nc.scalar.dma_start(hi, stage[N_FILT:128, :gw])

    nc.compile()
    return nc


def _gabor_weights(center_frequencies, bandwidths):
    """(w16 [128,256] fp16, w8 [128,256] e4m3) weight tiles, float64 math."""
    t = np.arange(-201, 201, dtype=np.float64)
    bw = np.asarray(bandwidths, dtype=np.float64)[:, None]
    cf = np.asarray(center_frequencies, dtype=np.float64)[:, None]
    env = np.exp(-(t**2) / (2.0 * bw**2)) / (np.sqrt(2.0 * np.pi) * bw)
    kre = (env * np.cos(cf * t)).T * SCALE  # [402, 64], scaled
    kim = (env * np.sin(cf * t)).T * SCALE
    W = np.concatenate([kre, kim], 1)       # [402, 128]

    w16 = np.zeros((128, 256), np.float16)
    w16[:, 0:128] = W[TAP16_A : TAP16_A + 128].astype(np.float16)
    w16[:, 128:256] = W[TAP16_B : TAP16_B + 128].astype(np.float16)

    w8 = np.zeros((128, 256), E4M3)
    w8[0:TAP16_A, 0:128] = W[0:TAP16_A].astype(E4M3)        # taps 0..64
    w8[1:82, 128:256] = W[321:402].astype(E4M3)             # taps 321..401
    return w16, w8


def _get_nc():
    global _CACHED_NC
    if _CACHED_NC is None:
        _CACHED_NC = _build()
    return _CACHED_NC


def kernel(x, center_frequencies, bandwidths):
    x = np.asarray(x, dtype=np.float32).reshape(16, T_IN)
    w16_host, w8_host = _gabor_weights(center_frequencies, bandwidths)

    # pad each row so the hankel view reaches x[p + i] (127+15919=16046)
    xp16 = np.zeros((16, W16H + 128), np.float16)
    xp16[:, :T_IN] = x.astype(np.float16)

    def spans_of(xp, row, spans, width, itemsize):
        v = np.lib.stride_tricks.as_strided(
            xp[row], shape=(128, width), strides=(itemsize, itemsize)
        )
        out, s0 = [], 0
        for w in spans:
            out.append(np.ascontiguousarray(v[:, s0 : s0 + w]))
            s0 += w
        return out

    nc = _get_nc()
    in_maps = []
    for i in range(N_CORES):
        m = {"wt16": w16_host, "wt8": w8_host}
        for r in range(ROWS_PER_CORE):
            row = i * ROWS_PER_CORE + r
            for j, a in enumerate(spans_of(xp16, row, SPANS16, W16H, 2)):
                m[f"hk16_{r}_{j}"] = a
        in_maps.append(m)

    br = run_bass_kernel_spmd(nc, in_maps, core_ids=list(range(N_CORES)))

    groups = _groups_of_row()
    inv = np.float32(1.0 / SCALE)
    out = np.empty((16, N_FILT, T_OUT), np.complex64)
    fl = np.empty((ROWS_PER_CORE, 128, T_OUT), np.float32)
    for i, r in enumerate(br.results):
        buf = r["o_fl"]
        for row in range(ROWS_PER_CORE):
            for g0, gw, _tiles in groups:
                base = row * 128 * T_OUT + 128 * g0
                fl[row, :, g0 : g0 + gw] = buf[base : base + 128 * gw].reshape(
                    128, gw
                )
        fl_s = fl * inv
        sl = slice(i * ROWS_PER_CORE, (i + 1) * ROWS_PER_CORE)
        out[sl].real = fl_s[:, :N_FILT]
        out[sl].imag = fl_s[:, N_FILT:]
    return out
